# revision 1
# baseline (speedup 1.0000x reference)
"""Trainium2 Bass kernel for nn_AnchorStores (retrieval_knn).

Per batch row b (one NeuronCore each, 8 cores data-parallel over B):
  dists[k] = mean_d qa[b,k,d] * (ln qa[b,k,d] - ln logits[b,d])   [K=256]
  top-8 over k, softmax, scatter into 4 classes by queue_label.

Device algorithm per core:
  - PE transposes qa 128x128 tiles into PSUM ([d_part, k_free] layout).
  - ACT computes t = Ln(qa_T) (optionally folding 1/logits via the
    per-partition `scale`), PSUM -> SBUF.
  - DVE computes u = qa_T * (t - ln logits[d]) either as a plain
    tensor_tensor multiply (strategy A: ACT already folded the log) or a
    fused scalar_tensor_tensor (strategy B).  The A/B split balances the
    ACT and DVE engines (both ~128us) under the ~144us HBM roofline.
  - PE reduces over d (partition axis) with a ones-vector matmul,
    accumulating into a [1, 512] PSUM tile across all 393 chunks.
  - Tiny epilogue: fold halves, scale by -1/(T*D), DVE max (top-8 values),
    threshold mask + exp + normalize, and two 1-column matmuls to scatter
    softmax weights into the 4 classes via a host-staged one-hot matrix.

Host prep is limited to small tensors: per-core ln(logits)/1/logits in a
[128, 393] partition-inner layout, a 128x128 identity, and the [128, 8]
one-hot label matrix.  The 412MB queue_anchor goes to the device untouched.
"""

import sys

for _p in ("/opt/trn_rl_repo",):
    if _p not in sys.path:
        sys.path.insert(0, _p)

import os

import numpy as np

B, K, DIM = 8, 256, 50257
KNN, N_CLASS = 8, 4
KNN_T = 0.05
CH = 128                      # d-chunk size (PE transpose tile)
NCH = (DIM + CH - 1) // CH    # 393 chunks, last one 81 wide
LAST_W = DIM - (NCH - 1) * CH # 81
TD = 4096                     # DMA tile width (cols of d) = 32 chunks
GSZ = 4                       # chunks per ACT/DVE batch group
# strategy A (ACT folds 1/logits) fraction ~0.7 balances ACT vs DVE
A_MOD, A_LIM = 10, int(os.environ.get("KNN_A_LIM", "7"))
QA_BUFS = int(os.environ.get("KNN_QA_BUFS", "2"))
TU_BUFS = int(os.environ.get("KNN_TU_BUFS", "4"))
PSG_BUFS = int(os.environ.get("KNN_PSG_BUFS", "2"))
LAG = int(os.environ.get("KNN_LAG", "2"))


def _group_is_a(g):
    return (g % A_MOD) < A_LIM


def build_nc(repeat=1):
    import concourse.bass as bass
    import concourse.tile as tile
    from concourse import bacc, mybir

    F32 = mybir.dt.float32
    F16 = mybir.dt.float16
    AF = mybir.ActivationFunctionType
    ALU = mybir.AluOpType

    nc = bacc.Bacc("TRN2", target_bir_lowering=False, debug=False, num_devices=8)
    qa_d = nc.dram_tensor("queue_anchor", [K, DIM], F32, kind="ExternalInput")
    inv_d = nc.dram_tensor("inv_pi", [CH, NCH], F32, kind="ExternalInput")
    ll_d = nc.dram_tensor("ll_pi", [CH, NCH], F32, kind="ExternalInput")
    id_d = nc.dram_tensor("ident", [CH, CH], F32, kind="ExternalInput")
    oh_d = nc.dram_tensor("lab_oh", [CH, 2 * N_CLASS], F32, kind="ExternalInput")
    out_d = nc.dram_tensor("out", [1, N_CLASS], F32, kind="ExternalOutput")

    n_dtiles = (DIM + TD - 1) // TD
    groups = []  # (first_chunk, n_chunks)
    c = 0
    while c < NCH:
        n = min(GSZ, NCH - c)
        # keep groups within one DMA tile (TD is a multiple of GSZ*CH)
        n = min(n, (c // (TD // CH) + 1) * (TD // CH) - c)
        groups.append((c, n))
        c += n
    total_pairs = sum((n + 1) // 2 for _, n in groups)

    with tile.TileContext(nc) as tc:
        with (
            tc.tile_pool(name="consts", bufs=1) as consts,
            tc.tile_pool(name="qa", bufs=QA_BUFS) as qa_pool,
            tc.tile_pool(name="tu", bufs=TU_BUFS) as tu_pool,
            tc.tile_pool(name="eps", bufs=1) as eps_pool,
            tc.tile_pool(name="psg", bufs=PSG_BUFS, space=bass.MemorySpace.PSUM) as ps_pool,
            tc.tile_pool(name="psd", bufs=1, space=bass.MemorySpace.PSUM) as psd_pool,
            tc.tile_pool(name="pse", bufs=1, space=bass.MemorySpace.PSUM) as pse_pool,
        ):
            inv_sb = consts.tile([CH, NCH], F32)
            nc.sync.dma_start(inv_sb[:], inv_d[:])
            ll_sb = consts.tile([CH, NCH], F32)
            nc.sync.dma_start(ll_sb[:], ll_d[:])
            id_sb = consts.tile([CH, CH], F32)
            nc.sync.dma_start(id_sb[:], id_d[:])
            oh_sb = consts.tile([CH, 2 * N_CLASS], F32)
            nc.sync.dma_start(oh_sb[:], oh_d[:])
            ones_sb = consts.tile([CH, 1], F32)
            nc.vector.memset(ones_sb[:], 1.0)
            ones16 = consts.tile([CH, 1], F16)
            nc.vector.memset(ones16[:], 1.0)

            def body():
                dists_ps = psd_pool.tile([1, 2 * K], F32)
                pair_idx = 0
                pending = []  # (u_tile, gn) reduces delayed LAG groups so PE
                              # never blocks on DVE before next group's transposes

                def emit_reduce(u_t, gn):
                    nonlocal pair_idx
                    for p0 in range(0, gn, 2):
                        pn = min(2, gn - p0) * K
                        nc.tensor.matmul(
                            dists_ps[0:1, 0:pn],
                            ones16[:, 0:1],
                            u_t[:, p0 * K:p0 * K + pn],
                            start=(pair_idx == 0),
                            stop=(pair_idx == total_pairs - 1),
                            skip_group_check=True,
                        )
                        pair_idx += 1

                for dt in range(n_dtiles):
                    off = dt * TD
                    w = min(TD, DIM - off)
                    qa_t0 = qa_pool.tile([CH, TD], F32, tag="qa0")
                    qa_t1 = qa_pool.tile([CH, TD], F32, tag="qa1")
                    nc.sync.dma_start(qa_t0[:, 0:w], qa_d[0:CH, off:off + w])
                    nc.sync.dma_start(qa_t1[:, 0:w], qa_d[CH:K, off:off + w])
                    for (g0, gn) in [g for g in groups if off <= g[0] * CH < off + w]:
                        gi = g0 // GSZ
                        gw = gn * K
                        ps_g = ps_pool.tile([CH, GSZ * K], F32, tag="psg")
                        if g0 + gn == NCH:  # group holds the 81-wide tail chunk
                            nc.vector.memset(
                                ps_g[:, (gn - 1) * K:gn * K], 1.0)
                        for ci in range(gn):
                            cg = g0 + ci           # global chunk id
                            cw = LAST_W if cg == NCH - 1 else CH
                            co = cg * CH - off     # col offset in qa tile
                            for kg, qat in ((0, qa_t0), (1, qa_t1)):
                                nc.tensor.matmul(
                                    ps_g[0:cw, ci * K + kg * CH:
                                         ci * K + kg * CH + CH],
                                    qat[:, co:co + cw],
                                    id_sb[:],
                                    is_transpose=True,
                                    start=True, stop=True,
                                    skip_group_check=True,
                                )
                        if len(pending) >= LAG:
                            emit_reduce(*pending.pop(0))
                        t_g = tu_pool.tile([CH, GSZ * K], F32, tag="t")
                        u_g = tu_pool.tile([CH, GSZ * K], F16, tag="u")
                        if _group_is_a(gi):
                            for ci in range(gn):
                                cg = g0 + ci
                                nc.scalar.activation(
                                    t_g[:, ci * K:(ci + 1) * K],
                                    ps_g[:, ci * K:(ci + 1) * K],
                                    AF.Ln,
                                    scale=inv_sb[:, cg:cg + 1],
                                )
                            nc.vector.tensor_mul(
                                u_g[:, 0:gw], t_g[:, 0:gw], ps_g[:, 0:gw])
                        else:
                            nc.scalar.activation(
                                t_g[:, 0:gw], ps_g[:, 0:gw], AF.Ln)
                            for ci in range(gn):
                                cg = g0 + ci
                                nc.vector.scalar_tensor_tensor(
                                    u_g[:, ci * K:(ci + 1) * K],
                                    t_g[:, ci * K:(ci + 1) * K],
                                    ll_sb[:, cg:cg + 1],
                                    ps_g[:, ci * K:(ci + 1) * K],
                                    op0=ALU.subtract,
                                    op1=ALU.mult,
                                )
                        pending.append((u_g, gn))
                for u_t, gn in pending:
                    emit_reduce(u_t, gn)

                # ---- epilogue (tiny) ----
                h0 = eps_pool.tile([1, K], F32, tag="h0")
                nc.vector.tensor_copy(h0[:], dists_ps[0:1, 0:K])
                h1 = eps_pool.tile([1, K], F32, tag="h1")
                nc.vector.tensor_add(h1[:], h0[:], dists_ps[0:1, K:2 * K])
                s_sb = eps_pool.tile([1, K], F32, tag="s")
                nc.vector.tensor_scalar_mul(s_sb[:], h1[:], -1.0 / (KNN_T * DIM))
                top8 = eps_pool.tile([1, 8], F32, tag="top8")
                nc.vector.max(top8[:], s_sb[:])
                negm = eps_pool.tile([1, 1], F32, tag="negm")
                nc.vector.tensor_scalar_mul(negm[:], top8[0:1, 0:1], -1.0)
                e_sb = eps_pool.tile([1, K], F32, tag="e")
                nc.scalar.activation(e_sb[:], s_sb[:], AF.Exp, bias=negm[0:1, 0:1])
                ge_sb = eps_pool.tile([1, K], F32, tag="ge")
                nc.vector.tensor_scalar(
                    ge_sb[:], s_sb[:], top8[0:1, 7:8], None, op0=ALU.is_ge)
                # unnormalized softmax weights; the host divides the [1,4]
                # output by its sum (tensor_tensor_reduce and reciprocal are
                # rejected by this runtime).
                w_sb = eps_pool.tile([1, K], F32, tag="w")
                nc.vector.tensor_mul(w_sb[:], e_sb[:], ge_sb[:])
                wcol_ps = pse_pool.tile([CH, 2], F32, tag="wcol")
                for g in (0, 1):
                    nc.tensor.matmul(
                        wcol_ps[:, g:g + 1],
                        w_sb[0:1, g * CH:(g + 1) * CH],
                        ones_sb[0:1, 0:1],
                        start=True, stop=True, skip_group_check=True)
                wcol_sb = eps_pool.tile([CH, 2], F32, tag="wcs")
                nc.vector.tensor_copy(wcol_sb[:], wcol_ps[:])
                prob_ps = pse_pool.tile([1, N_CLASS], F32, tag="prob")
                for g in (0, 1):
                    nc.tensor.matmul(
                        prob_ps[:],
                        wcol_sb[:, g:g + 1],
                        oh_sb[:, g * N_CLASS:(g + 1) * N_CLASS],
                        start=(g == 0), stop=(g == 1), skip_group_check=True)
                out_sb = eps_pool.tile([1, N_CLASS], F32, tag="osb")
                nc.vector.tensor_copy(out_sb[:], prob_ps[:])
                nc.sync.dma_start(out_d[:], out_sb[:])

            if repeat == 1:
                body()
            else:
                with tc.For_i(0, repeat, 1):
                    body()

    nc.compile()
    return nc


def host_prep(logits_b, label_b):
    """Small per-core host-side tensors derived from logits/labels."""
    pad = np.ones(NCH * CH, np.float32)
    pad[:DIM] = logits_b
    arr = pad.reshape(NCH, CH).T          # [128, NCH]; arr[p, i] = pad[128*i+p]
    inv_pi = np.ascontiguousarray(1.0 / arr)
    ll_pi = np.ascontiguousarray(np.log(arr))
    oh = np.zeros((CH, 2 * N_CLASS), np.float32)
    lab = np.asarray(label_b).astype(np.int64)
    for g in (0, 1):
        oh[np.arange(CH), N_CLASS * g + lab[g * CH:(g + 1) * CH]] = 1.0
    return inv_pi, ll_pi, oh


def make_in_maps(logits, queue_anchor, queue_label):
    logits = np.asarray(logits, dtype=np.float32)
    qa = np.asarray(queue_anchor, dtype=np.float32)
    ident = np.eye(CH, dtype=np.float32)
    in_maps = []
    for b in range(B):
        inv_pi, ll_pi, oh = host_prep(logits[b], np.asarray(queue_label)[b])
        in_maps.append({
            "queue_anchor": np.ascontiguousarray(qa[b]),
            "inv_pi": inv_pi,
            "ll_pi": ll_pi,
            "ident": ident,
            "lab_oh": oh,
        })
    return in_maps


_NC = None


def kernel(logits, queue_anchor, queue_label):
    global _NC
    from concourse.bass_utils import run_bass_kernel_spmd

    if _NC is None:
        _NC = build_nc(repeat=1)
    in_maps = make_in_maps(logits, queue_anchor, queue_label)
    res = run_bass_kernel_spmd(_NC, in_maps, core_ids=list(range(B)))
    out = np.stack([np.asarray(res.results[i]["out"][0]) for i in range(B)])
    out = out / out.sum(axis=1, keepdims=True)
    return out.astype(np.float32)


if __name__ == "__main__":
    rng = np.random.default_rng(0)
    inputs = {
        "logits": rng.uniform(1e-3, 1.0, (B, DIM)).astype(np.float32),
        "queue_anchor": rng.uniform(1e-3, 1.0, (B, K, DIM)).astype(np.float32),
        "queue_label": rng.integers(0, N_CLASS, (B, K)).astype(np.int32),
    }
    out = kernel(**inputs)
    print(out)

